# Initial kernel scaffold
#
"""Chamfer distance (K=1 squared-euclidean NN, both directions) on 8
Trainium2 NeuronCores.

Sharding: 8 independent work units = 4 batches x 2 directions; one unit per
core (SPMD — same program, different inputs). Per unit: queries Q[8192,3]
vs keys K[8192,3].

Device algorithm per unit:
  u[p,q] = 2*q_p.k_q - ||q_p||^2 - ||k_q||^2 = -(d^2)     via K=5 matmul
    with augmented operands lhsT = [2qx,2qy,2qz,||q||^2,1],
                            rhs  = [kx,ky,kz,-1,-||k||^2].
  * TensorE: K=5/M=128/N=512 matmuls fill [128,2048] 4-bank PSUM supertiles
    (two supertiles ping-pong). Per-block weights are staged to a fixed SBUF
    slot by DMA (walrus forbids register offsets in matmul weights).
  * VectorE: ONE 1x-rate pass — a running prefix-max scan
    (tensor_tensor_scan, op=max, carry chained across supertiles) PSUM ->
    SBUF. M = last scan element = exact max of u; cham = -M (negated on
    host).
  * ScalarE: argmax (first occurrence) via a counting trick:
    sign(M - scan_q) is +1 exactly for q < q_first and 0 after, so one
    activation(Sign, scale=-1, bias=M, accum_out) yields idx directly as an
    exact fp32 integer (int-cast on host).
  * Sync engine (SP): stages weights and streams per-block results (M, idx)
    to DRAM with dynamic-offset DMAs.

The whole program runs inside hardware Fori loops (pair-of-blocks bodies so
buffers alternate with static APs): this environment charges a large fixed
cost per *unique* instruction, so the program is ~130 instructions
re-executed via branches. Cross-engine sync is credit-based: semaphores +
per-waiter credit registers (wait_ge takes a register), with
`nop().then_inc(sem, n)` priming standing in for negative initial credits.
"""

from contextlib import ExitStack

import numpy as np

import concourse.bass as bass
import concourse.mybir as mybir
from concourse.bass import ds
from concourse.bass_utils import run_bass_kernel_spmd

F32 = mybir.dt.float32
NEG_BIG = -3.0e38

N_BATCH = 4
NPTS = 8192
N_CORES = 8


def build_chamfer_bass(P1=NPTS, P2=NPTS, repeat=1):
    """Single-core Bass program (SPMD across cores)."""
    sup = 2048                       # keys per supertile (4 PSUM banks)
    assert P1 % 256 == 0 and P2 % (2 * sup) == 0
    qb = P1 // 128                   # query blocks
    pairs = qb // 2                  # 2 blocks per loop iteration
    nsup = P2 // sup                 # supertiles per block (even)
    ntile = sup // 512
    assert nsup % 2 == 0

    nc = bass.Bass()
    qka = nc.dram_tensor("qka", [5, P1 + P2], F32, kind="ExternalInput")
    mval = nc.dram_tensor("mval", [128, qb], F32, kind="ExternalOutput")
    idxf = nc.dram_tensor("idxf", [128, qb], F32, kind="ExternalOutput")

    with ExitStack() as ctx:
        ec = ctx.enter_context
        keys_sb = ec(nc.sbuf_tensor([5, P2], F32))
        wstage = ec(nc.sbuf_tensor([5, 256], F32))  # A: 0:128, B: 128:256
        dummy = ec(nc.sbuf_tensor([128, sup], F32))
        scan_a = ec(nc.sbuf_tensor([128, P2], F32))
        scan_b = ec(nc.sbuf_tensor([128, P2], F32))
        junk = ec(nc.sbuf_tensor([128, P2], mybir.dt.bfloat16))
        m1 = ec(nc.sbuf_tensor([128, 2], F32))      # per-half M
        a1 = ec(nc.sbuf_tensor([128, 2], F32))      # per-half idx (float)
        ps_a = ec(nc.psum_tensor([128, sup], F32))
        ps_b = ec(nc.psum_tensor([128, sup], F32))
        s_dma = ec(nc.semaphore("s_dma"))
        s_stage = ec(nc.semaphore("s_stage"))
        s_mm = ec(nc.semaphore("s_mm"))
        s_scan = ec(nc.semaphore("s_scan"))
        s_act = ec(nc.semaphore("s_act"))
        s_ext = ec(nc.semaphore("s_ext"))
        s_out = ec(nc.semaphore("s_out"))
        block = ec(nc.Block())
        ps = [ps_a, ps_b]
        scans = [scan_a, scan_b]

        @block.sync
        def _(sync):
            sync.dma_start(out=keys_sb[:, :], in_=qka[:, P1:P1 + P2]
                           ).then_inc(s_dma, 16)
            sync.dma_start(
                out=dummy[:, :],
                in_=bass.AP(tensor=qka, offset=0, ap=[[0, 128], [1, sup]]),
            ).then_inc(s_dma, 16)
            qoff = sync.alloc_register("qoff")
            mreg = sync.alloc_register("mreg")
            r_pe = sync.alloc_register("r_pe")
            r_sg = sync.alloc_register("r_sg")
            r_sa = sync.alloc_register("r_sa")
            sync.reg_mov(r_pe, 0)
            sync.reg_mov(r_sg, 0)
            sync.reg_mov(r_sa, 2)

            def stage_pair():
                qv = sync.snap(qoff, min_val=0, max_val=P1)
                sync.dma_start(out=wstage[:, 0:128],
                               in_=qka[:, ds(qv, 128)]).then_inc(s_stage, 16)
                sync.dma_start(out=wstage[:, 128:256],
                               in_=qka[:, ds(qv + 128, 128)]
                               ).then_inc(s_stage, 16)
                sync.reg_add(qoff, qoff, 256)

            def outs_block(half):
                # results of global block (mreg) are in slot `half`
                sync.reg_add(r_sa, r_sa, 1)
                sync.wait_ge(s_act, r_sa)
                mv = sync.snap(mreg, min_val=0, max_val=qb - 1)
                with nc.allow_non_contiguous_dma(
                        reason="128 scattered 4B column writes per block"):
                    sync.dma_start(out=mval[:, ds(mv, 1)],
                                   in_=m1[:, half:half + 1]
                                   ).then_inc(s_out, 16)
                    sync.dma_start(out=idxf[:, ds(mv, 1)],
                                   in_=a1[:, half:half + 1]
                                   ).then_inc(s_out, 16)
                sync.reg_add(mreg, mreg, 1)

            with sync.Fori(0, repeat, 1):
                sync.reg_mov(qoff, 0)
                sync.reg_mov(mreg, 0)
                # previous rep's staging fully drained (wstage WAW)
                sync.wait_ge(s_stage, r_sg)
                sync.reg_add(r_sg, r_sg, 32 * (pairs + 1))
                stage_pair()                      # pair 0
                if pairs > 1:
                    with sync.Fori(0, pairs - 1, 1):
                        # stage pair i+1 once pair i's matmuls are done
                        sync.reg_add(r_pe, r_pe, 2 * nsup)
                        sync.wait_ge(s_mm, r_pe)
                        stage_pair()
                        # stream out pair i's results (signs lag matmuls
                        # by about a block, so they don't stall staging)
                        outs_block(0)
                        outs_block(1)
                sync.reg_add(r_pe, r_pe, 2 * nsup)  # last pair's matmuls
                sync.wait_ge(s_mm, r_pe)
                stage_pair()                      # garbage pre-stage
                outs_block(0)
                outs_block(1)
            sync.wait_ge(s_out, 64 * pairs * repeat)
            sync.wait_ge(s_dma, 32)

        @block.tensor
        def _(tensor):
            tensor.wait_ge(s_dma, 32)
            r_stage = tensor.alloc_register("r_stage")
            r_scan = tensor.alloc_register("r_scan")
            tensor.reg_mov(r_stage, 0)
            tensor.reg_mov(r_scan, 0)
            with tensor.Fori(0, repeat, 1):
                with tensor.Fori(0, pairs, 1):
                    # pair staged (both DMAs; completion order arbitrary)
                    tensor.reg_add(r_stage, r_stage, 32)
                    tensor.wait_ge(s_stage, r_stage)
                    for half in range(2):
                        lhsT = wstage[:, 128 * half:128 * half + 128]
                        for s in range(nsup):
                            # psum slot free: scan of its previous use done
                            # (s_scan primed with 2 credits)
                            tensor.reg_add(r_scan, r_scan, 1)
                            tensor.wait_ge(s_scan, r_scan)
                            last = None
                            for t in range(ntile):
                                last = nc.tensor.matmul(
                                    ps[s % 2][:, 512 * t:512 * (t + 1)],
                                    lhsT,
                                    keys_sb[:, sup * s + 512 * t:
                                            sup * s + 512 * (t + 1)],
                                    start=True, stop=True,
                                )
                            last.then_inc(s_mm, 1)
                # swallow the end-of-rep garbage staging batch's credits
                tensor.reg_add(r_stage, r_stage, 32)

        @block.vector
        def _(vector):
            vector.wait_ge(s_dma, 32)
            vector.nop().then_inc(s_scan, 2)      # psum-reuse priming
            r_mm = vector.alloc_register("r_mm")
            r_act = vector.alloc_register("r_act")
            r_self = vector.alloc_register("r_self")
            vector.reg_mov(r_mm, 0)
            vector.reg_mov(r_act, 0)
            vector.reg_mov(r_self, 2)
            with vector.Fori(0, pairs * repeat, 1):
                for half in range(2):
                    sb = scans[half]
                    # scanbuf slot free: sign of its previous user done
                    # (s_act primed with 2 credits)
                    vector.reg_add(r_act, r_act, 1)
                    vector.wait_ge(s_act, r_act)
                    for s in range(nsup):
                        vector.reg_add(r_mm, r_mm, 1)
                        vector.wait_ge(s_mm, r_mm)
                        if s > 0:
                            # carry element committed
                            vector.wait_ge(s_scan, r_self)
                        init = (NEG_BIG if s == 0
                                else sb[:, sup * s - 1:sup * s])
                        nc.vector.tensor_tensor_scan(
                            out=sb[:, sup * s:sup * (s + 1)],
                            data0=ps[s % 2][:, :],
                            data1=dummy[:, :],
                            initial=init,
                            op0=mybir.AluOpType.max,
                            op1=mybir.AluOpType.bypass,
                        ).then_inc(s_scan, 1)
                        vector.reg_add(r_self, r_self, 1)

        @block.scalar
        def _(scalar):
            scalar.nop().then_inc(s_act, 2)       # scanbuf-reuse priming
            r_scan = scalar.alloc_register("r_scan")
            r_ext = scalar.alloc_register("r_ext")
            r_jw = scalar.alloc_register("r_jw")
            r_so = scalar.alloc_register("r_so")
            scalar.reg_mov(r_scan, 2)
            scalar.reg_mov(r_ext, 0)
            scalar.reg_mov(r_jw, 1)
            scalar.reg_mov(r_so, 0)
            with scalar.Fori(0, pairs * repeat, 1):
                # sync's copy-outs of the previous pair (this pair's slots'
                # previous users) must be done; their completion order is
                # arbitrary so wait at pair granularity
                scalar.wait_ge(s_out, r_so)
                scalar.reg_add(r_so, r_so, 64)
                for half in range(2):
                    sb = scans[half]
                    scalar.reg_add(r_scan, r_scan, nsup)
                    scalar.wait_ge(s_scan, r_scan)
                    nc.scalar.activation(
                        out=m1[:, half:half + 1], in_=sb[:, P2 - 1:P2],
                        func=mybir.ActivationFunctionType.Copy, scale=1.0,
                    ).then_inc(s_ext, 1)
                    # extract committed (bias RAW)
                    scalar.reg_add(r_ext, r_ext, 1)
                    scalar.wait_ge(s_ext, r_ext)
                    # previous sign committed (junk WAW; s_act primed +2)
                    scalar.reg_add(r_jw, r_jw, 1)
                    scalar.wait_ge(s_act, r_jw)
                    # sign(M - scan_q): +1 before first argmax, 0 after;
                    # accum = first-occurrence argmax as exact fp32 int
                    nc.scalar.activation(
                        out=junk[:, :], in_=sb[:, :],
                        func=mybir.ActivationFunctionType.Sign,
                        bias=m1[:, half:half + 1], scale=-1.0,
                        accum_out=a1[:, half:half + 1],
                    ).then_inc(s_act, 1)

    return nc


def make_unit_inputs(q, k):
    """Host-side augmentation for one (query cloud, key cloud) unit."""
    q = np.ascontiguousarray(q, np.float32)
    k = np.ascontiguousarray(k, np.float32)
    p1, p2 = q.shape[0], k.shape[0]
    qka = np.empty((5, p1 + p2), np.float32)
    qka[0:3, :p1] = 2.0 * q.T
    qka[3, :p1] = (q * q).sum(-1, dtype=np.float32)
    qka[4, :p1] = 1.0
    qka[0:3, p1:] = k.T
    qka[3, p1:] = -1.0
    qka[4, p1:] = -((k * k).sum(-1, dtype=np.float32))
    return {"qka": qka}


_BUILT = {}


def _built_nc():
    if "nc" not in _BUILT:
        _BUILT["nc"] = build_chamfer_bass()
    return _BUILT["nc"]


def kernel(x, y, _collect_results=None):
    """Full-input entry point. x, y: (4, 8192, 3) float32.

    Returns (cham_x, cham_y, idx_x, idx_y) matching reference()."""
    x = np.asarray(x, np.float32)
    y = np.asarray(y, np.float32)
    n = x.shape[0]
    units = []
    in_maps = []
    for b in range(n):
        for d in range(2):
            q, k = (x[b], y[b]) if d == 0 else (y[b], x[b])
            in_maps.append(make_unit_inputs(q, k))
            units.append((b, d))
    nc = _built_nc()
    res = run_bass_kernel_spmd(nc, in_maps, core_ids=list(range(N_CORES)))
    if _collect_results is not None:
        _collect_results.append(res)
    cham_x = np.empty((n, x.shape[1]), np.float32)
    cham_y = np.empty((n, y.shape[1]), np.float32)
    idx_x = np.empty((n, x.shape[1]), np.int32)
    idx_y = np.empty((n, y.shape[1]), np.int32)
    for (b, d), r in zip(units, res.results):
        chamv = (-np.asarray(r["mval"])).T.reshape(-1)
        idxv = np.asarray(r["idxf"]).T.reshape(-1).astype(np.int32)
        if d == 0:
            cham_x[b], idx_x[b] = chamv, idxv
        else:
            cham_y[b], idx_y[b] = chamv, idxv
    return cham_x, cham_y, idx_x, idx_y



# revision 5
# speedup vs baseline: 1.1588x; 1.1588x over previous
"""Chamfer distance (K=1 squared-euclidean NN, both directions) on 8
Trainium2 NeuronCores — v2.

Sharding: 8 independent work units = 4 batches x 2 directions; one unit per
core (SPMD). Per unit: queries Q[8192,3] vs keys K[8192,3].

Device algorithm per unit (per 128-query block):
  u[p,q] = 2*q_p.k_q - ||q_p||^2 - ||k_q||^2 = -(d^2)
  * TensorE: bf16 matmuls with split-float (hi/lo) operands, K=13 rows:
      r0-2:  (2q)_hi . k_hi      r3-5: (2q)_hi . k_lo
      r6-8:  (2q)_lo . k_hi      r9,10: nq_hi/nq_lo . (-1)
      r11,12: 1 . (-nk_hi/-nk_lo)
    Exact bf16 products accumulate in fp32 PSUM; |u| error ~1e-4.
    bf16 runs at 1 cyc/col (fp32 was 2 passes x 4 cyc/col): 8x less PE.
  * ScalarE: copies each PSUM supertile [128,2048] to SBUF as bf16
    (u = -d2 is small near the max, so bf16 keeps cham error ~d2_min/512).
  * VectorE: chunk-16 max tree over ubuf [128,8192] bf16 (tensor_tensor max
    at 2x bf16 rate) -> cmax [128,512]; max8 -> M; max_index -> first chunk
    jstar achieving M.
  * GpSimd: two indirect_copy gathers (the gather ucode stages its data
    slab in Q7 DRAM; a full 16KB/partition row overflows it, 8KB halves
    fit), each fetching the lane's chunk candidate from one ubuf half with
    clamped indices; lane p's own chunk lands at slot p%16.
  * VectorE: scalar_tensor_tensor masks each half by (own-window mask) x
    (per-lane half-selector from jstar), then one max_index with needle M
    over the combined [128,512] stage -> position t.
    Host: idx = 16*jstar + ((t mod 256) - 16*(p%16)).
  * Sync engine: stages weights, streams per-block (mval, jstar, t) to DRAM.

Engines self-sync with semaphores + per-waiter credit registers; hardware
Fori loops over pairs of blocks (static APs via parity unrolling).
"""

from contextlib import ExitStack

import numpy as np
import ml_dtypes

import concourse.bass as bass
import concourse.bacc as bacc
import concourse.mybir as mybir
from concourse.bass import ds
from concourse.bass_utils import run_bass_kernel_spmd

F32 = mybir.dt.float32
BF16 = mybir.dt.bfloat16
F16 = mybir.dt.float16
U32 = mybir.dt.uint32
U16 = mybir.dt.uint16
I16 = mybir.dt.int16
ALU = mybir.AluOpType

N_BATCH = 4
NPTS = 8192
N_CORES = 8
CHUNK = 16                       # bf16 elems per gatherable chunk
SUP = 2048                       # keys per PSUM supertile (4 banks)


def build_chamfer_bass(P1=NPTS, P2=NPTS):
    """Single-core Bass program (SPMD across cores)."""
    assert P1 % 256 == 0 and P2 % (2 * SUP) == 0
    qb = P1 // 128                   # query blocks
    pairs = qb // 2                  # 2 blocks per loop iteration
    nsup = P2 // SUP                 # supertiles per block
    ntile = SUP // 512               # matmuls per supertile
    nch = P2 // CHUNK                # chunks per row
    hch = nch // 2                   # chunks per half-row
    hu32 = P2 // 4                   # u32 per half-row
    n_ops = 10                       # DVE ops per block (s_v incs)
    UNROLL = min(4, pairs)           # pairs per hw-loop iteration

    nc = bacc.Bacc("TRN2", target_bir_lowering=False)
    qk = nc.dram_tensor("qk", [13, P1 + P2], BF16, kind="ExternalInput")
    cst = nc.dram_tensor("cst", [128, 16 * CHUNK], F16, kind="ExternalInput")
    mval = nc.dram_tensor("mval", [128, qb], F32, kind="ExternalOutput")
    jst = nc.dram_tensor("jst", [128, qb], U32, kind="ExternalOutput")
    tst = nc.dram_tensor("tst", [128, qb], U32, kind="ExternalOutput")

    with ExitStack() as ctx:
        ec = ctx.enter_context
        keys4 = ec(nc.sbuf_tensor([128, P2], BF16))
        wstages = [ec(nc.sbuf_tensor(f"wstage{i}", [128, 256], BF16))
                   for i in range(2)]  # pair-parity slots
        maskc = ec(nc.sbuf_tensor([128, 16 * CHUNK], F16))
        ubufs = [ec(nc.sbuf_tensor(f"ubuf{i}", [128, P2], F16)) for i in range(2)]
        t1 = ec(nc.sbuf_tensor([128, P2 // 2], F16))
        t2 = ec(nc.sbuf_tensor([128, P2 // 4], F16))
        t3 = ec(nc.sbuf_tensor([128, P2 // 8], F16))
        cmax = ec(nc.sbuf_tensor([128, nch], F16))
        m8s = [ec(nc.sbuf_tensor(f"m8_{i}", [128, 8], F16)) for i in range(2)]
        j8s = [ec(nc.sbuf_tensor(f"j8_{i}", [128, 8], U32)) for i in range(2)]
        t8s = [ec(nc.sbuf_tensor(f"t8_{i}", [128, 8], U32)) for i in range(2)]
        cidxs = [ec(nc.sbuf_tensor(f"cidx{i}", [128, 1], I16)) for i in range(2)]
        stages = [ec(nc.sbuf_tensor(f"stage{i}", [128, 16 * CHUNK], F16)) for i in range(2)]
        mstage = ec(nc.sbuf_tensor([128, 16 * CHUNK], F16))
        mvs = [ec(nc.sbuf_tensor(f"mv{i}", [128, 1], F32)) for i in range(2)]
        ps = [ec(nc.psum_tensor(f"ps{i}", [128, SUP], F32)) for i in range(2)]
        s_dma = ec(nc.semaphore("s_dma"))
        s_stgs = [ec(nc.semaphore(f"s_stg{i}")) for i in range(2)]
        s_mm = ec(nc.semaphore("s_mm"))
        s_cp = ec(nc.semaphore("s_cp"))
        s_v = ec(nc.semaphore("s_v"))
        s_st = ec(nc.semaphore("s_st"))
        s_out = ec(nc.semaphore("s_out"))
        block = ec(nc.Block())

        @block.sync
        def _(sync):
            for g in range(4):
                sync.dma_start(out=keys4[32 * g:32 * g + 13, :],
                               in_=qk[:, P1:P1 + P2]).then_inc(s_dma, 16)
            sync.dma_start(out=maskc[:, :], in_=cst[:, :]).then_inc(s_dma, 16)
            sync.nop().then_inc(s_out, 96)       # mval/j8/t8 WAW priming
            qoff = sync.alloc_register("qoff")
            breg = sync.alloc_register("breg")
            r_pe = sync.alloc_register("r_pe")
            r_v9 = sync.alloc_register("r_v9")
            sync.reg_mov(qoff, 0)
            sync.reg_mov(breg, 0)
            sync.reg_mov(r_pe, 0)
            sync.reg_mov(r_v9, 2 * n_ops)       # 14b+14 + 14 priming, b=0

            def stage_pair(par):
                qv = sync.snap(qoff, min_val=0, max_val=P1)
                for g in range(4):
                    sync.dma_start(
                        out=wstages[par][32 * g:32 * g + 13, 0:128],
                        in_=qk[:, ds(qv, 128)]).then_inc(s_stgs[par], 16)
                    sync.dma_start(
                        out=wstages[par][32 * g:32 * g + 13, 128:256],
                        in_=qk[:, ds(qv + 128, 128)]).then_inc(s_stgs[par], 16)
                sync.reg_add(qoff, qoff, 256)

            def outs_block(half):
                # results of global block (breg) are in parity slot `half`
                sync.wait_ge(s_v, r_v9)
                sync.reg_add(r_v9, r_v9, n_ops)
                bv = sync.snap(breg, min_val=0, max_val=qb - 1)
                with nc.allow_non_contiguous_dma(
                        reason="128 scattered 4B column writes per block"):
                    sync.dma_start(out=mval[:, ds(bv, 1)],
                                   in_=mvs[half][:, :]).then_inc(s_out, 16)
                    sync.dma_start(out=jst[:, ds(bv, 1)],
                                   in_=j8s[half][:, 0:1]).then_inc(s_out, 16)
                    sync.dma_start(out=tst[:, ds(bv, 1)],
                                   in_=t8s[half][:, 0:1]).then_inc(s_out, 16)
                sync.reg_add(breg, breg, 1)

            stage_pair(0)                     # pair 0
            with sync.Fori(0, pairs // UNROLL, 1):
                for _u in range(UNROLL):
                    # stage pair j+1 (parity (j+1)%2) once pair j-1's
                    # matmuls are done (double-buffered: overlaps pair j)
                    sync.wait_ge(s_mm, r_pe)
                    sync.reg_add(r_pe, r_pe, 2 * nsup)
                    stage_pair((_u + 1) % 2)
                    outs_block(0)
                    outs_block(1)
            sync.wait_ge(s_out, 48 * qb + 96)
            sync.wait_ge(s_dma, 80)

        @block.tensor
        def _(tensor):
            tensor.wait_ge(s_dma, 80)
            r_stg = [tensor.alloc_register(f"r_stg{i}") for i in range(2)]
            r_cp = tensor.alloc_register("r_cp")
            tensor.reg_mov(r_stg[0], 128)
            tensor.reg_mov(r_stg[1], 128)
            tensor.reg_mov(r_cp, 1)
            with tensor.Fori(0, pairs // UNROLL, 1):
              for _u in range(UNROLL):
                # pair staged (per-parity sem: stagings overlap matmuls)
                _par = _u % 2
                tensor.wait_ge(s_stgs[_par], r_stg[_par])
                tensor.reg_add(r_stg[_par], r_stg[_par], 128)
                for half in range(2):
                    for st in range(nsup):
                        # psum slot free: copy of its previous user done
                        # (s_cp primed with 2 credits by scalar)
                        tensor.wait_ge(s_cp, r_cp)
                        tensor.reg_add(r_cp, r_cp, 1)
                        last = None
                        for t in range(ntile):
                            # 4 concurrent matmuls in distinct 32-row
                            # groups of the PE array (K=13 <= 32); each
                            # streams its own 512-key slice
                            last = nc.tensor.matmul(
                                ps[st % 2][:, 512 * t:512 * (t + 1)],
                                wstages[_par][32 * t:32 * t + 13,
                                              128 * half:
                                              128 * half + 128],
                                keys4[32 * t:32 * t + 13,
                                      SUP * st + 512 * t:
                                      SUP * st + 512 * (t + 1)],
                                start=True, stop=True,
                                tile_position=(32 * t, 0),
                            )
                        last.then_inc(s_mm, 1)

        @block.scalar
        def _(scalar):
            scalar.nop().then_inc(s_cp, 2)       # psum-reuse priming
            scalar.wait_ge(s_dma, 80)
            r_mm = scalar.alloc_register("r_mm")
            r_st = scalar.alloc_register("r_st")
            scalar.reg_mov(r_mm, 1)
            scalar.reg_mov(r_st, 0)

            def act_block(half):
                # ubuf(par) free: gather of block b-2 done (s_st primed +1)
                scalar.wait_ge(s_st, r_st)
                scalar.reg_add(r_st, r_st, 1)
                for st in range(nsup):
                    scalar.wait_ge(s_mm, r_mm)
                    scalar.reg_add(r_mm, r_mm, 1)
                    nc.scalar.activation(
                        out=ubufs[half][:, SUP * st:SUP * (st + 1)],
                        in_=ps[st % 2][:, :],
                        func=mybir.ActivationFunctionType.Copy, scale=1.0,
                    ).then_inc(s_cp, 1)

            with scalar.Fori(0, pairs // UNROLL, 1):
                for _u in range(UNROLL):
                    act_block(0)
                    act_block(1)

        @block.vector
        def _(vector):
            # slot-0 garbage convert reads m8s[1]: init it; the commit
            # carries the first 5 priming credits (total priming = 14)
            nc.vector.memset(m8s[1][:, :], 0).then_inc(s_v, n_ops)
            vector.wait_ge(s_dma, 80)
            r_cp = vector.alloc_register("r_cp")
            r_sv = vector.alloc_register("r_sv")
            r_out = vector.alloc_register("r_out")
            r_st = vector.alloc_register("r_st")
            vector.reg_mov(r_cp, nsup + 2)
            vector.reg_mov(r_sv, n_ops)
            vector.reg_mov(r_out, 96)
            vector.reg_mov(r_st, 2)

            def step(ins):
                ins.then_inc(s_v, 1)
                vector.reg_add(r_sv, r_sv, 1)
                vector.wait_ge(s_v, r_sv)

            def vec_block(half):
                ub = ubufs[half]
                # all copies of this block landed
                vector.wait_ge(s_cp, r_cp)
                vector.reg_add(r_cp, r_cp, nsup)
                # chunk-16 max tree: L1 pairs (e, e+8) within chunks
                step(nc.vector.tensor_tensor(              # op 1
                    out=bass.AP(tensor=t1, offset=0,
                                ap=[[P2 // 2, 128], [8, nch], [1, 8]]),
                    in0=bass.AP(tensor=ub, offset=0,
                                ap=[[P2, 128], [16, nch], [1, 8]]),
                    in1=bass.AP(tensor=ub, offset=8,
                                ap=[[P2, 128], [16, nch], [1, 8]]),
                    op=ALU.max,
                ))
                step(nc.vector.tensor_tensor(              # op 2
                    out=bass.AP(tensor=t2, offset=0,
                                ap=[[P2 // 4, 128], [4, nch], [1, 4]]),
                    in0=bass.AP(tensor=t1, offset=0,
                                ap=[[P2 // 2, 128], [8, nch], [1, 4]]),
                    in1=bass.AP(tensor=t1, offset=4,
                                ap=[[P2 // 2, 128], [8, nch], [1, 4]]),
                    op=ALU.max,
                ))
                step(nc.vector.tensor_tensor(              # op 3
                    out=bass.AP(tensor=t3, offset=0,
                                ap=[[P2 // 8, 128], [2, nch], [1, 2]]),
                    in0=bass.AP(tensor=t2, offset=0,
                                ap=[[P2 // 4, 128], [4, nch], [1, 2]]),
                    in1=bass.AP(tensor=t2, offset=2,
                                ap=[[P2 // 4, 128], [4, nch], [1, 2]]),
                    op=ALU.max,
                ))
                step(nc.vector.tensor_tensor(              # op 4
                    out=cmax[:, :],
                    in0=bass.AP(tensor=t3, offset=0, ap=[[P2 // 8, 128], [2, nch]]),
                    in1=bass.AP(tensor=t3, offset=1, ap=[[P2 // 8, 128], [2, nch]]),
                    op=ALU.max,
                ))
                # m8/j8/t8/mval parity slots free? (pair-granular)
                vector.wait_ge(s_out, r_out)
                if half == 1:
                    vector.reg_add(r_out, r_out, 96)
                step(nc.vector.max(out=m8s[half][:, :], in_=cmax[:, :]))  # 5
                step(nc.vector.max_index(                  # op 6
                    out=j8s[half][:, :], in_max=m8s[half][:, :],
                    in_values=cmax[:, :]))
                # chunk index for the gather (uint32 -> int16)
                step(nc.vector.tensor_scalar(              # op 7
                    out=cidxs[half][:, :], in0=j8s[half][:, 0:1],
                    scalar1=1.0, scalar2=None, op0=ALU.mult,
                ))
                # mval convert (bf16 -> fp32), keeps ScalarE copy-only
                step(nc.vector.tensor_copy(                # op 8
                    out=mvs[half][:, :], in_=m8s[half][:, 0:1]))
                # stage gathered
                vector.wait_ge(s_st, r_st)
                vector.reg_add(r_st, r_st, 1)
                # additive mask: own window +0.0 (exact), foreign -1e9
                # (mult-by-0 masking would collide with legitimate u==0.0
                # maxima from near-duplicate points)
                step(nc.vector.tensor_tensor(              # op 7
                    out=mstage[:, :],
                    in0=stages[half][:, :],
                    in1=maskc[:, :],
                    op=ALU.add,
                ))
                step(nc.vector.max_index(                  # op 8
                    out=t8s[half][:, :], in_max=m8s[half][:, :],
                    in_values=mstage[:, :]))

            with vector.Fori(0, pairs // UNROLL, 1):
                for _u in range(UNROLL):
                    vec_block(0)
                    vec_block(1)

        @block.gpsimd
        def _(gpsimd):
            gpsimd.nop().then_inc(s_st, 1)       # ubuf-reuse priming
            r_v6 = gpsimd.alloc_register("r_v6")
            gpsimd.reg_mov(r_v6, n_ops + 7)      # 10b+7 + 10 priming, b=0

            def gp_block(half):
                gpsimd.wait_ge(s_v, r_v6)
                gpsimd.reg_add(r_v6, r_v6, n_ops)
                nc.gpsimd.ap_gather(
                    out_ap=bass.AP(tensor=stages[half], offset=0,
                                   ap=[[16 * CHUNK, 128], [CHUNK, 16],
                                       [1, CHUNK]]),
                    in_ap=bass.AP(tensor=ubufs[half], offset=0,
                                  ap=[[P2, 128], [CHUNK, nch], [1, CHUNK]]),
                    idxs_ap=cidxs[half][:, :],
                    channels=128, num_elems=nch, d=CHUNK, num_idxs=16,
                ).then_inc(s_st, 1)

            with gpsimd.Fori(0, pairs // UNROLL, 1):
                for _u in range(UNROLL):
                    gp_block(0)
                    gp_block(1)

    nc.compile()
    return nc


def _split_bf16(a):
    """Split float32 array into bf16 hi + lo parts."""
    hi = a.astype(ml_dtypes.bfloat16)
    lo = (a - hi.astype(np.float32)).astype(ml_dtypes.bfloat16)
    return hi, lo


def make_unit_inputs(q, k):
    """Host-side augmentation for one (query cloud, key cloud) unit."""
    q = np.ascontiguousarray(q, np.float32)
    k = np.ascontiguousarray(k, np.float32)
    p1, p2 = q.shape[0], k.shape[0]
    qk = np.empty((13, p1 + p2), ml_dtypes.bfloat16)
    a_hi, a_lo = _split_bf16(2.0 * q.T)            # (3, p1)
    nq_hi, nq_lo = _split_bf16((q * q).sum(-1, dtype=np.float32))
    k_hi, k_lo = _split_bf16(k.T)                  # (3, p2)
    nk_hi, nk_lo = _split_bf16((k * k).sum(-1, dtype=np.float32))
    # queries (lhsT columns)
    qk[0:3, :p1] = a_hi
    qk[3:6, :p1] = a_hi
    qk[6:9, :p1] = a_lo
    qk[9, :p1] = nq_hi
    qk[10, :p1] = nq_lo
    qk[11, :p1] = 1.0
    qk[12, :p1] = 1.0
    # keys (rhs columns)
    qk[0:3, p1:] = k_hi
    qk[3:6, p1:] = k_lo
    qk[6:9, p1:] = k_hi
    qk[9, p1:] = -1.0
    qk[10, p1:] = -1.0
    qk[11, p1:] = -nk_hi.astype(np.float32)
    qk[12, p1:] = -nk_lo.astype(np.float32)
    return {"qk": qk, "cst": make_mask()}


def make_mask():
    mask = np.full((128, 16 * CHUNK), -60000.0, dtype=np.float16)
    for w in range(16):
        mask[w::16, CHUNK * w:CHUNK * w + CHUNK] = 0.0
    return mask


_BUILT = {}


def _built_nc():
    if "nc" not in _BUILT:
        _BUILT["nc"] = build_chamfer_bass()
    return _BUILT["nc"]


def combine_outputs(r, p1):
    """Host-side: r = {mval, jst, tst} [128, qb] -> (cham, idx) flat (p1,)."""
    mv = np.asarray(r["mval"], np.float32)
    j = np.asarray(r["jst"]).astype(np.int64)
    t = np.asarray(r["tst"]).astype(np.int64)
    pmod = (np.arange(128) % 16)[:, None]
    idx = CHUNK * j + (t - CHUNK * pmod)
    cham = (-mv).T.reshape(-1)[:p1]
    idxv = idx.T.reshape(-1)[:p1].astype(np.int32)
    return cham, idxv


def kernel(x, y, _collect_results=None):
    """Full-input entry point. x, y: (4, 8192, 3) float32.

    Returns (cham_x, cham_y, idx_x, idx_y) matching reference()."""
    x = np.asarray(x, np.float32)
    y = np.asarray(y, np.float32)
    n = x.shape[0]
    units = []
    in_maps = []
    for b in range(n):
        for d in range(2):
            q, k = (x[b], y[b]) if d == 0 else (y[b], x[b])
            in_maps.append(make_unit_inputs(q, k))
            units.append((b, d))
    nc = _built_nc()
    res = run_bass_kernel_spmd(nc, in_maps, core_ids=list(range(N_CORES)))
    if _collect_results is not None:
        _collect_results.append(res)
    cham_x = np.empty((n, x.shape[1]), np.float32)
    cham_y = np.empty((n, y.shape[1]), np.float32)
    idx_x = np.empty((n, x.shape[1]), np.int32)
    idx_y = np.empty((n, y.shape[1]), np.int32)
    for (b, d), r in zip(units, res.results):
        chamv, idxv = combine_outputs(r, x.shape[1])
        if d == 0:
            cham_x[b], idx_x[b] = chamv, idxv
        else:
            cham_y[b], idx_y[b] = chamv, idxv
    return cham_x, cham_y, idx_x, idx_y
